# revision 8
# baseline (speedup 1.0000x reference)
"""Trainium2 Bass kernel for batched weighted scatter-add (AttentionCopy).

Computes out[b, o, v] = sum_i attn[b, o, i] * (ids[b, i] == v)
for ids [16, 512] int32 in [0, 50000), attn [16, 32, 512] f32,
out [16, 32, 50000] f32.

Strategy: pure data parallel over the batch dim — 2 batches per core on 8
cores. Per batch the [32, 50000] output is built densely in 10 PSUM tiles of
[128, 1250], one per contiguous vocab span of 5000 = 4 groups x 1250. Tile
rows are o-major (o, gl) pairs (o in 0..31, gl in 0..3 local group), so each
tile's DRAM write is a [32, 4, 1250] access pattern whose outer dim (32)
spreads across all 16 SDMA engines (outer-dim count < 16 would leave engines
idle — measured 4x DMA slowdown with a g-major [4, 32, 1250] pattern).

The host buckets each batch's 512 ids into the 10 spans (index-only
preprocessing; uniform ids put ~51 of 512 in each span, max 67 observed,
capacity 128) and gathers the matching attn columns, so the device does a
single K=128 matmul pass per tile instead of K=512 over all ids:

  out[(o, gl), lo] = gt.T @ alo,   gt[i, (o, gl)] = (hi_i == gl) * attnT[i, o]
                                   alo[i, lo]     = (lo_i == lo)

with hi/lo the div/mod-1250 split of the span-relative id (sentinel -1 for
padding slots; its gt column is all zero). This cuts tensor-engine time ~4x
(it was the bottleneck at 53us busy of 65us total), leaving the kernel
bounded by the mandatory 12.8 MB/core f32 output write.

All 20 alo/gt one-hot builds are issued up front on the vector engine (they
only depend on the inputs), so the steady-state loop is a pure
matmul -> PSUM-copy -> DMA pipeline; copies go to scalar early (vector is
still building) and alternate scalar/vector once the builds are done.
Output DMAs alternate the two HWDGE queues (scalar + sync).

hi = x // 1250 uses the round-to-nearest int cast of
(x + 0.5) * (1/1250) - 0.5, validated exhaustively on HW for [0, 50000).
"""

import sys

sys.path.insert(0, "/opt/trn_rl_repo")

import numpy as np

NCORES = 8
B, O, I = 16, 32, 512
SIZE = 50000
BPC = B // NCORES  # batches per core
V2 = 1250  # lo range (one output tile is 2.5 PSUM banks)
GPT = 4  # groups per output tile: 128 rows = 32 o x 4 groups
SPAN = GPT * V2  # 5000: vocab span per output tile
TILES = SIZE // SPAN  # 10 output tiles per batch
KW = 128  # id-window capacity per (batch, tile)
NW = BPC * TILES  # 20 windows per core
# matmul N-slices of V2, each within one 2 KiB PSUM bank
NSLICES = [(0, 512), (512, 1024), (1024, 1250)]
NWARM = 16  # tensor-engine warmup matmuls (DVFS clock ramp)
LA = 4  # one-hot build lookahead (tiles)

_cache = {}


def _build(mm_dtype="bfloat16", nwarm=NWARM):
    import concourse.bacc as bacc
    import concourse.mybir as mybir
    import concourse.tile as tile

    f32 = mybir.dt.float32
    f16 = mybir.dt.float16
    mmdt = getattr(mybir.dt, mm_dtype)
    i32 = mybir.dt.int32
    Alu = mybir.AluOpType

    nc = bacc.Bacc("TRN2", target_bir_lowering=False, debug=False, num_devices=NCORES)

    # ids pre-bucketed on host to [128, NW]: [p, b*TILES+t] = span-relative id
    # of slot p in batch b's window for output tile t (-1 = empty slot)
    ids_d = nc.dram_tensor("ids", [128, NW], i32, kind="ExternalInput").ap()
    # attn columns gathered to match: [b, p, t*O+o] = attn[b, o, orig_i(b,t,p)]
    attn_d = nc.dram_tensor("attn", [BPC, 128, TILES * O], f32, kind="ExternalInput").ap()
    gidx_d = nc.dram_tensor("gidx", [128, O * GPT], f16, kind="ExternalInput").ap()
    lov_d = nc.dram_tensor("lov", [128, V2], f16, kind="ExternalInput").ap()
    out_d = nc.dram_tensor("out", [BPC, O, SIZE], f32, kind="ExternalOutput").ap()

    with tile.TileContext(nc) as tc:
        with (
            tc.tile_pool(name="const", bufs=1) as constp,
            tc.tile_pool(name="idx", bufs=1) as idxp,
            tc.tile_pool(name="gt", bufs=LA + 2) as gtp,
            tc.tile_pool(name="alo", bufs=LA + 2) as alop,
            tc.tile_pool(name="outs", bufs=8) as outp,
            tc.tile_pool(name="psmm", bufs=2, space="PSUM") as psmm,
        ):
            if nwarm:
                warm = constp.tile([128, 256], mmdt, tag="warm")
                nc.gpsimd.memset(warm[:], 0)
                wps = psmm.tile([128, 256], f32, tag="wm", bufs=1)
                for _ in range(nwarm):
                    nc.tensor.matmul(out=wps[:, :256], lhsT=warm[:, :128],
                                     rhs=warm[:, :256], start=True, stop=True)

            lov = constp.tile([128, V2], f16, tag="lov")
            nc.sync.dma_start(out=lov[:], in_=lov_d[:])
            gidx = constp.tile([128, O * GPT], f16, tag="gidx")
            nc.scalar.dma_start(out=gidx[:], in_=gidx_d[:])
            ids_all = idxp.tile([128, NW], i32, tag="ids_all")
            nc.scalar.dma_start(out=ids_all[:], in_=ids_d[:])
            at = []
            for b in range(BPC):
                t_ = constp.tile([128, TILES * O], f32, tag=f"attn{b}", name=f"at{b}")
                nc.sync.dma_start(out=t_[:], in_=attn_d[b])
                at.append(t_)

            # hi = x // 1250 via RTN int cast of (x+0.5)/1250 - 0.5
            # (exact for [0, 50000) incl. sentinel -1); lo = x - 1250*hi
            ids_f = idxp.tile([128, NW], f32, tag="ids_f")
            nc.vector.tensor_copy(out=ids_f[:], in_=ids_all[:])
            tq = idxp.tile([128, NW], f32, tag="tq")
            nc.vector.tensor_scalar(out=tq[:], in0=ids_f[:], scalar1=0.5,
                                    scalar2=float(np.float32(1.0 / V2)),
                                    op0=Alu.add, op1=Alu.mult)
            hi_i = idxp.tile([128, NW], i32, tag="hi_i")
            nc.vector.tensor_scalar(out=hi_i[:], in0=tq[:], scalar1=0.5,
                                    scalar2=None, op0=Alu.subtract)
            hi_f = idxp.tile([128, NW], f32, tag="hi_f")
            nc.vector.tensor_copy(out=hi_f[:], in_=hi_i[:])
            lo_f = idxp.tile([128, NW], f32, tag="lo_f")
            nc.vector.scalar_tensor_tensor(out=lo_f[:], in0=hi_f[:],
                                           scalar=float(-V2), in1=ids_f[:],
                                           op0=Alu.mult, op1=Alu.add)

            # one-hot builds run LA tiles ahead of the matmuls, interleaved
            # with the vector engine's share of the PSUM->SBUF copies, so
            # the matmul -> copy -> DMA pipeline starts immediately and the
            # vector engine is never a serial prefix
            alos, gts = [], []

            def build(w):
                b, t = divmod(w, TILES)
                alo = alop.tile([128, V2], mmdt, tag="alo", name=f"alo{w}")
                nc.vector.tensor_scalar(out=alo[:], in0=lov[:],
                                        scalar1=lo_f[:, w : w + 1],
                                        scalar2=None, op0=Alu.is_equal)
                gt = gtp.tile([128, O * GPT], mmdt, tag="gt", name=f"gt{w}")
                nc.vector.scalar_tensor_tensor(
                    out=gt[:].rearrange("p (o g) -> p o g", g=GPT),
                    in0=gidx[:].rearrange("p (o g) -> p o g", g=GPT),
                    scalar=hi_f[:, w : w + 1],
                    in1=at[b][:, t * O : (t + 1) * O]
                    .unsqueeze(2)
                    .broadcast_to([128, O, GPT]),
                    op0=Alu.is_equal,
                    op1=Alu.mult,
                )
                alos.append(alo)
                gts.append(gt)

            for w in range(LA):
                build(w)

            for w in range(NW):
                if w + LA < NW:
                    build(w + LA)
                b, t = divmod(w, TILES)
                alo, gt = alos[w], gts[w]
                ps = psmm.tile([128, V2], f32, tag="mm")
                for n0, n1 in NSLICES:
                    nc.tensor.matmul(out=ps[:, n0:n1], lhsT=gt[:],
                                     rhs=alo[:, n0:n1], start=True, stop=True)
                os_ = outp.tile([128, V2], f32, tag="os")
                # [32, 4, 1250] view; iteration order (o, g, l) matches
                # the SBUF tile's (partition=(o,g), l) order, and the
                # outer dim of 32 spreads over all 16 SDMA engines
                outv = out_d[b][:, t * SPAN : (t + 1) * SPAN].rearrange(
                    "o (g l) -> o g l", l=V2
                )
                if w == NW - 1:
                    # tail: split halves across engines/queues
                    h = V2 // 2
                    nc.scalar.copy(out=os_[:, :h], in_=ps[:, :h])
                    nc.vector.tensor_copy(out=os_[:, h:], in_=ps[:, h:])
                    nc.scalar.dma_start(out=outv[:, :, :h], in_=os_[:, :h])
                    nc.sync.dma_start(out=outv[:, :, h:], in_=os_[:, h:])
                elif w % 2 == 0:
                    nc.scalar.copy(out=os_[:], in_=ps[:])
                    nc.scalar.dma_start(out=outv, in_=os_[:])
                else:
                    nc.vector.tensor_copy(out=os_[:], in_=ps[:])
                    nc.sync.dma_start(out=outv, in_=os_[:])

    nc.compile()
    return nc


def _consts():
    gidx = np.broadcast_to(
        np.tile(np.arange(GPT, dtype=np.float16), O)[None, :], (128, O * GPT)
    ).copy()
    lov = np.broadcast_to(
        np.arange(V2, dtype=np.float16)[None, :], (128, V2)
    ).copy()
    return gidx, lov


def _in_maps(ids, attn):
    gidx, lov = _consts()
    ids_w = np.full((B, TILES, KW), -1, dtype=np.int32)
    attn_w = np.zeros((B, TILES, KW, O), dtype=np.float32)
    for b in range(B):
        t_of = ids[b] // SPAN
        for t in range(TILES):
            sel = np.nonzero(t_of == t)[0]
            c = sel.size
            if c > KW:
                raise RuntimeError(
                    f"id window overflow: batch {b} span {t} has {c} > {KW} ids"
                )
            ids_w[b, t, :c] = ids[b, sel] - t * SPAN
            attn_w[b, t, :c, :] = attn[b][:, sel].T
    ids_t = ids_w.reshape(NCORES, NW, KW).transpose(0, 2, 1)  # [8, 128, NW]
    attn_t = attn_w.reshape(NCORES, BPC, TILES, KW, O).transpose(
        0, 1, 3, 2, 4
    ).reshape(NCORES, BPC, KW, TILES * O)
    in_maps = [
        {
            "ids": np.ascontiguousarray(ids_t[c]),
            "attn": np.ascontiguousarray(attn_t[c]),
            "gidx": gidx,
            "lov": lov,
        }
        for c in range(NCORES)
    ]
    return in_maps


def kernel(ids, attn):
    from concourse.bass_utils import run_bass_kernel_spmd

    ids = np.ascontiguousarray(ids, dtype=np.int32)
    attn = np.ascontiguousarray(attn, dtype=np.float32)

    if "nc" not in _cache:
        _cache["nc"] = _build()
    nc = _cache["nc"]

    core_ids = list(range(NCORES))
    res = run_bass_kernel_spmd(nc, _in_maps(ids, attn), core_ids)
    out = np.concatenate([res.results[c]["out"] for c in core_ids], axis=0)
    return out
